# revision 1
# baseline (speedup 1.0000x reference)
"""ConvTranspose2d(64->64, k=3, s=1, p=0) on (2, 64, 1024, 1024) fp32.

out[b, o, p, q] = sum_{c,kh,kw} weight[c, o, kh, kw] * x[b, c, p-kh, q-kw]
out shape (2, 64, 1026, 1026).

Strategy (8 NeuronCores, pure data parallel over batch x H):
  - Each core handles one batch and a quarter of the output rows.
  - Output rows are processed in PAIRS (2j, 2j+1). Input rows are stacked
    in pairs U_j = [x[2j]; x[2j+1]] as SBUF tiles of 128 partitions
    (partition = 64*u + c, u = row-of-pair, c = channel).
  - Per output pair and kw-shift s, two K=128 matmuls accumulate in PSUM:
      A_s^T @ U_j     with A_s = [[W0s, W1s], [0, W0s]]
      B_s^T @ U_{j-1} with B_s = [[W2s, 0], [W1s, W2s]]
    where Wks = weight[:, :, k, s]. Output partition = 64*v + o (v =
    out-row-of-pair). 1026 output columns are split in 3 PSUM chunks of 342.
  - Matmuls run as float32r (TF32-class, full PE rate at N>=256), PSUM
    accumulates fp32. Column shifts use 2 zero pad columns on each side of
    the 1028-wide row tiles (pads baked into the host-packed input).
"""

import numpy as np

B = 2
C = 64
H = 1024
W = 1024
HO = 1026
WO = 1026
WP = W + 4  # 2 zero pad cols each side
NPAIR = 129  # output row pairs computed per core
NTILE = NPAIR + 1  # U tiles per core incl. leading halo tile
J0S = (0, 128, 256, 384)  # first output pair per core (within a batch)
VALID = (128, 128, 128, 129)  # pairs consumed from each core
CHUNKS = ((0, 342), (342, 342), (684, 342))

U_BUFS = 6
O_BUFS = 4
PS_BUFS = 2

_CACHE = {}


def _build(npair=NPAIR, reps=1, u_bufs=U_BUFS, o_bufs=O_BUFS, ps_bufs=PS_BUFS):
    import concourse.bacc as bacc
    import concourse.mybir as mybir
    from concourse.tile import TileContext

    F32 = mybir.dt.float32
    F32R = mybir.dt.float32r

    nc = bacc.Bacc()
    xs = nc.dram_tensor("xs", [npair + 1, 128, WP], F32R, kind="ExternalInput")
    ws = nc.dram_tensor("ws", [128, 768], F32R, kind="ExternalInput")
    outs = nc.dram_tensor("outs", [npair, 128, WO], F32, kind="ExternalOutput")
    with TileContext(nc) as tc:
        with (
            tc.tile_pool(name="w", bufs=1) as wp,
            tc.tile_pool(name="u", bufs=u_bufs) as up,
            tc.tile_pool(name="ob", bufs=o_bufs) as ob,
            tc.tile_pool(name="ps", bufs=ps_bufs, space="PSUM") as pp,
        ):
            wsb = wp.tile([128, 768], F32R)
            nc.sync.dma_start(out=wsb, in_=ws[:, :])
            for _ in range(reps):
                prev = None
                for j in range(npair):
                    if prev is None:
                        prev = up.tile([128, WP], F32R, tag="u")
                        nc.sync.dma_start(out=prev, in_=xs[0])
                    cur = up.tile([128, WP], F32R, tag="u")
                    nc.sync.dma_start(out=cur, in_=xs[j + 1])
                    osb = ob.tile([128, WO], F32, tag="ob")
                    for ci, (n0, nch) in enumerate(CHUNKS):
                        ps = pp.tile([128, nch], F32, tag=f"c{ci}")
                        k = 0
                        for g, ut in ((0, cur), (1, prev)):
                            for s in range(3):
                                i0 = (g * 3 + s) * 128
                                nc.tensor.matmul(
                                    ps[:, :],
                                    wsb[:, i0 : i0 + 128],
                                    ut[:, n0 + 2 - s : n0 + 2 - s + nch],
                                    start=(k == 0),
                                    stop=(k == 5),
                                )
                                k += 1
                        nc.vector.tensor_copy(out=osb[:, n0 : n0 + nch], in_=ps[:, :])
                    nc.sync.dma_start(out=outs[j], in_=osb)
                    prev = cur
    nc.compile()
    return nc


def _pack_weight(weight):
    """weight (64, 64, 3, 3) fp32 -> (128, 768) stationary blocks.

    ws[64*u + c, (3*g + s)*128 + 64*v + o] = weight[c, o, v - u + 2*g, s]
    when 0 <= v - u + 2*g <= 2 else 0.
    """
    wsb = np.zeros((128, 768), np.float32)
    for g in (0, 1):
        for s in range(3):
            col0 = (3 * g + s) * 128
            for u in (0, 1):
                for v in (0, 1):
                    kh = v - u + 2 * g
                    if 0 <= kh <= 2:
                        wsb[64 * u : 64 * u + 64, col0 + 64 * v : col0 + 64 * v + 64] = (
                            weight[:, :, kh, s]
                        )
    return wsb


def _pack_core_input(xb, j0):
    """xb (64, 1024, 1024) fp32 -> xs (130, 128, 1028) for pairs j0..j0+128.

    xs[t] holds U_{j0+t-1}: rows 2*(j0+t-1) and +1, zero outside [0, H),
    with 2 zero pad columns on both sides.
    """
    xs = np.zeros((NTILE * 2, 64, WP), np.float32)
    r0 = 2 * j0 - 2  # first source row
    lo = max(0, r0)
    hi = min(H, r0 + 2 * NTILE)
    xs[lo - r0 : hi - r0, :, 2 : 2 + W] = xb[:, lo:hi, :].transpose(1, 0, 2)
    return xs.reshape(NTILE, 128, WP)


def kernel(x, weight):
    from concourse.bass_utils import run_bass_kernel_spmd

    x = np.ascontiguousarray(x, dtype=np.float32)
    weight = np.ascontiguousarray(weight, dtype=np.float32)

    if "nc" not in _CACHE:
        _CACHE["nc"] = _build()
    nc = _CACHE["nc"]

    wsb = _pack_weight(weight)
    in_maps = []
    for core in range(8):
        b, k = divmod(core, 4)
        in_maps.append({"xs": _pack_core_input(x[b], J0S[k]), "ws": wsb})

    res = run_bass_kernel_spmd(nc, in_maps, core_ids=list(range(8)))

    out = np.empty((B, C, HO, WO), np.float32)
    for core in range(8):
        b, k = divmod(core, 4)
        nv = VALID[k]
        rows = res.results[core]["outs"].reshape(NPAIR * 2, C, WO)
        out[b, :, 2 * J0S[k] : 2 * (J0S[k] + nv), :] = rows[: 2 * nv].transpose(1, 0, 2)
    return out



# revision 17
# speedup vs baseline: 49.2502x; 49.2502x over previous
"""ConvTranspose2d(64->64, k=3, s=1, p=0) on (2, 64, 1024, 1024) fp32.

out[b, o, p, q] = sum_{c,kh,kw} weight[c, o, kh, kw] * x[b, c, p-kh, q-kw]
out shape (2, 64, 1026, 1026).

Strategy (8 NeuronCores, pure data parallel over batch x H):
  - Each core handles one batch and a quarter of the output rows.
  - Output rows are processed in PAIRS (2j, 2j+1). Input rows are stacked
    in pairs U_j = [x[2j]; x[2j+1]] as SBUF partitions 64*u + c.
  - Per output pair and kw-shift s, two K=128 matmuls accumulate in PSUM:
      A_s^T @ U_j     with A_s = [[W0s, W1s], [0, W0s]]
      B_s^T @ U_{j-1} with B_s = [[W2s, 0], [W1s, W2s]]
    where Wks = weight[:, :, k, s]. Output partition = 64*v + o (v =
    out-row-of-pair). 1026 output columns are split in 3 PSUM chunks of 342.
  - All data moves as bf16 (fp32 PSUM accumulate; rel err ~3e-3, budget
    2e-2). Halves HBM traffic vs fp32.
  - DMAs are batched as SUPER-tiles of SUPER row-pairs (~1 MB each) and
    all kept on ONE HWDGE ring (nc.sync). Concurrent rings interleave
    HBM reads and writes at packet granularity and collapse to ~75 GB/s
    each; a single ring runs each transfer at its solo ~300 GB/s. Large
    transfers amortize the ~1 us per-DMA HBM write-receipt stall.
  - Column shifts use 2 zero pad columns on each side of the 1028-wide
    row images (pads baked into the host-packed input).
"""

import numpy as np
import ml_dtypes

BF16 = np.dtype(ml_dtypes.bfloat16)

B = 2
C = 64
H = 1024
W = 1024
HO = 1026
WO = 1026
WP = W + 4  # 2 zero pad cols each side
SUPER = 3  # row-pairs per super-tile / per DMA
WSUP = SUPER * WP  # input super-tile free size
OSUP = SUPER * WO  # output super-tile free size
NPAIR = 129  # output row pairs computed per core (>= 129 needed)
NSUPO = NPAIR // SUPER  # output super-tiles per core
NSUPI = NSUPO + 1  # input super-tiles per core (incl. halo)
NROWS = 2 * SUPER * NSUPI  # packed input rows per core
J0S = (0, 128, 256, 384)  # first output pair per core (within a batch)
VALID = (128, 128, 128, 129)  # pairs consumed from each core
CHUNKS = ((0, 342), (342, 342), (684, 342))

U_BUFS = 6
O_BUFS = 3
PS_BUFS = 8  # single shared PSUM ring, one bank per buffer
PREFETCH = 3  # input supers requested ahead of use
WARMUP_MM = 12  # dummy matmuls to flip the HAM clock gate before real work

_CACHE = {}


def _build(npair=NPAIR, reps=1, u_bufs=U_BUFS, o_bufs=O_BUFS, ps_bufs=PS_BUFS):
    import concourse.bacc as bacc
    import concourse.mybir as mybir
    from concourse.tile import TileContext

    F32 = mybir.dt.float32
    BF = mybir.dt.bfloat16
    nsupo = npair // SUPER
    nsupi = nsupo + 1

    nc = bacc.Bacc()
    xs = nc.dram_tensor("xs", [nsupi, 128, WSUP], BF, kind="ExternalInput")
    ws = nc.dram_tensor("ws", [128, 768], BF, kind="ExternalInput")
    outs = nc.dram_tensor("outs", [nsupo, 128, OSUP], BF, kind="ExternalOutput")
    with TileContext(nc) as tc:
        with (
            tc.tile_pool(name="w", bufs=1) as wp,
            tc.tile_pool(name="u", bufs=u_bufs) as up,
            tc.tile_pool(name="ob", bufs=o_bufs) as ob,
            tc.tile_pool(name="ps", bufs=ps_bufs, space="PSUM") as pp,
        ):
            wsb = wp.tile([128, 768], BF)
            # weights first, split so the warmup matmuls below only wait
            # for the first half (~120KB, lands ~1us after ring-open)
            nc.sync.dma_start(out=wsb[:, :470], in_=ws[:, :470])
            nc.sync.dma_start(out=wsb[:, 470:], in_=ws[:, 470:])
            # PE pre-warm: ~4us of dummy matmuls flip the HAM clock gate
            # to 8/8 while the rest of the weights + first supers load
            wps = pp.tile([128, 342], F32, tag="c")
            for _ in range(WARMUP_MM):
                nc.tensor.matmul(
                    wps[:, :], wsb[:, 0:128], wsb[:, 128:470], start=True, stop=True
                )
            for _ in range(reps):
                sups = [
                    up.tile([128, WSUP], BF, tag="u", name=f"sup{i}")
                    for i in range(2)
                ]
                # split the first super load so pair 0 only waits for its
                # two row-pairs, not the whole tile
                nc.sync.dma_start(out=sups[0][:, : 2 * WP], in_=xs[0, :, : 2 * WP])
                nc.sync.dma_start(out=sups[0][:, 2 * WP :], in_=xs[0, :, 2 * WP :])
                nc.sync.dma_start(out=sups[1], in_=xs[1])
                for i in range(2, min(PREFETCH + 1, nsupi)):
                    pre0 = up.tile([128, WSUP], BF, tag="u", name=f"sup{i}")
                    nc.sync.dma_start(out=pre0, in_=xs[i])
                    sups.append(pre0)
                for n in range(nsupo):
                    sup_prev, sup_next = sups[n], sups[n + 1]
                    if n + PREFETCH + 1 < nsupi:
                        pre = up.tile([128, WSUP], BF, tag="u")
                        nc.sync.dma_start(out=pre, in_=xs[n + PREFETCH + 1])
                        sups.append(pre)
                    osb = ob.tile([128, OSUP], BF, tag="ob")
                    for h in range(SUPER):
                        # local pair i = SUPER*n + h: prev = U_i, cur = U_{i+1}
                        prev_t, prev_o = sup_prev, h * WP
                        if h == SUPER - 1:
                            cur_t, cur_o = sup_next, 0
                        else:
                            cur_t, cur_o = sup_prev, (h + 1) * WP
                        for ci, (n0, nch) in enumerate(CHUNKS):
                            ps = pp.tile([128, nch], F32, tag="c")
                            k = 0
                            for g, (ut, uo) in (
                                (0, (cur_t, cur_o)),
                                (1, (prev_t, prev_o)),
                            ):
                                for s in range(3):
                                    i0 = (g * 3 + s) * 128
                                    c0 = uo + n0 + 2 - s
                                    nc.tensor.matmul(
                                        ps[:, :],
                                        wsb[:, i0 : i0 + 128],
                                        ut[:, c0 : c0 + nch],
                                        start=(k == 0),
                                        stop=(k == 5),
                                    )
                                    k += 1
                            nc.vector.tensor_copy(
                                out=osb[:, h * WO + n0 : h * WO + n0 + nch],
                                in_=ps[:, :],
                            )
                    # outputs on the ACT ring: an out-DMA's sem wait on its
                    # casts would convoy-block queued input loads on sync.
                    # The last super is written per-pair so the final DMA
                    # after the last cast is ~1/3 size (shorter tail).
                    if n == nsupo - 1:
                        for h in range(SUPER):
                            nc.scalar.dma_start(
                                out=outs[n, :, h * WO : (h + 1) * WO],
                                in_=osb[:, h * WO : (h + 1) * WO],
                            )
                    else:
                        nc.scalar.dma_start(out=outs[n], in_=osb)
    nc.compile()
    return nc


def _pack_weight(weight):
    """weight (64, 64, 3, 3) fp32 -> (128, 768) bf16 stationary blocks.

    ws[64*u + c, (3*g + s)*128 + 64*v + o] = weight[c, o, v - u + 2*g, s]
    when 0 <= v - u + 2*g <= 2 else 0.
    """
    wsb = np.zeros((128, 768), np.float32)
    for g in (0, 1):
        for s in range(3):
            col0 = (3 * g + s) * 128
            for u in (0, 1):
                for v in (0, 1):
                    kh = v - u + 2 * g
                    if 0 <= kh <= 2:
                        wsb[64 * u : 64 * u + 64, col0 + 64 * v : col0 + 64 * v + 64] = (
                            weight[:, :, kh, s]
                        )
    return wsb.astype(BF16)


def _pack_core_input(xb16, j0):
    """xb16 (64, 1024, 1024) bf16 -> xs (NSUPI, 128, WSUP) bf16 supers.

    Super s, partition 64*u + c, free h*WP + q holds
    x[c, 2*(j0-1) + 2*SUPER*s + 2*h + u, q-2] (zero outside bounds).
    """
    rows = np.zeros((NROWS, 64, WP), BF16)
    r0 = 2 * j0 - 2
    lo = max(0, r0)
    hi = min(H, r0 + NROWS)
    rows[lo - r0 : hi - r0, :, 2 : 2 + W] = xb16[:, lo:hi, :].transpose(1, 0, 2)
    xs = (
        rows.reshape(NSUPI, SUPER, 2, 64, WP)
        .transpose(0, 2, 3, 1, 4)
        .reshape(NSUPI, 128, WSUP)
    )
    return np.ascontiguousarray(xs)


def _unpack_core_output(arr, nv):
    """arr (NSUPO, 128, OSUP) bf16 -> (64, 2*nv, 1026) fp32 rows."""
    rows = (
        arr.reshape(NSUPO, 2, 64, SUPER, WO)
        .transpose(2, 0, 3, 1, 4)
        .reshape(64, NSUPO * 2 * SUPER, WO)
    )
    return rows[:, : 2 * nv, :].astype(np.float32)


def kernel(x, weight):
    from concourse.bass_utils import run_bass_kernel_spmd

    x16 = np.ascontiguousarray(x, dtype=np.float32).astype(BF16)
    weight = np.ascontiguousarray(weight, dtype=np.float32)

    if "nc" not in _CACHE:
        _CACHE["nc"] = _build()
    nc = _CACHE["nc"]

    wsb = _pack_weight(weight)
    in_maps = []
    for core in range(8):
        b, k = divmod(core, 4)
        in_maps.append({"xs": _pack_core_input(x16[b], J0S[k]), "ws": wsb})

    res = run_bass_kernel_spmd(nc, in_maps, core_ids=list(range(8)))

    out = np.empty((B, C, HO, WO), np.float32)
    for core in range(8):
        b, k = divmod(core, 4)
        nv = VALID[k]
        out[b, :, 2 * J0S[k] : 2 * J0S[k] + 2 * nv, :] = _unpack_core_output(
            res.results[core]["outs"], nv
        )
    return out
